# revision 16
# baseline (speedup 1.0000x reference)
"""Blenderbot cross-attention on 8 Trainium2 NeuronCores.

Sharding: 2D tensor-parallel — 4 head-groups (8 heads = 512 proj cols each)
x 2 batch-pairs (2 batches = 2048 tokens each). Each core computes its
Q/K/V column shard for its batch pair, attention for its (heads x batches)
block, and a row-parallel partial of the output projection. The host sums
the 4 head-group partials per batch pair (RowParallel unshard) and adds bo.

Device layout trick: the host pre-transposes x/xa to [d, tok], so the
Q/K projections emit Q^T/K^T directly (head-dim on partitions — the layout
attention's contraction needs), V emits natural [tok, cols] (the layout the
probs@V contraction needs), and attn^T feeds the O-projection as lhsT.
Zero on-chip transposes. kv_len == cache len, so the "cache update" outputs
are just the K/V projections; cache_k/cache_v inputs are never read.

All matmuls run as float32r (FP22, full PE rate at N>=256, ~1e-4 rel err).
Softmax: exp on ScalarE with fused 1/8 scale + per-partition mask bias
(no max-subtraction needed: |scores| << 88); denominator via a fused
ones-column in the probs@V matmul (M=65); normalization by reciprocal +
gpsimd partition-broadcast + one vector multiply per (b, h, q-tile).
"""

import sys

for _p in ("/opt/trn_rl_repo",):
    if _p not in sys.path:
        sys.path.insert(0, _p)

import numpy as np

import concourse.bass as bass
import concourse.tile as tile
from concourse import bacc, mybir
from concourse.bass_utils import run_bass_kernel_spmd

# Problem constants (hardcoded per contract)
D = 2048          # d_model
H_TOT = 32        # total heads
HD = 64           # head dim
B_TOT = 4         # total batch
S = 1024          # q seq len == kv seq len
P = 128

# Per-core shard
GROUPS = 4        # head groups
PAIRS = 2         # batch pairs
HEADS = 8         # heads per core
COLS = HEADS * HD # 512 projection cols per core
NB = 2            # batches per core
TOK = NB * S      # 2048 tokens per core

DCH = D // P      # 16 contraction chunks for projections
CCH = COLS // P   # 4 col chunks
TCH = TOK // P    # 16 token chunks
KVCH = S // P     # 8 kv chunks per batch
QT = 512          # q tile (free dim of scores/attn matmuls)
NQT = S // QT     # 2 q tiles per batch
TT = 256          # token tile for projections
NTT = TOK // TT   # 8 token tiles

F32 = mybir.dt.float32
F32R = mybir.dt.float32r
EXP = mybir.ActivationFunctionType.Exp
IDENT = mybir.ActivationFunctionType.Identity
MULT = mybir.AluOpType.mult

_compiled = None  # cached (nc,) so repeat kernel() calls skip rebuild


def build_nc(reps=1):
    nc = bacc.Bacc("TRN2", target_bir_lowering=False, debug=False, num_devices=8)

    # DRAM I/O (per-core shapes)
    xT = nc.dram_tensor("xT", [D, TOK], F32, kind="ExternalInput").ap()
    xaT = nc.dram_tensor("xaT", [D, TOK], F32, kind="ExternalInput").ap()
    wqT = nc.dram_tensor("wqT", [D, COLS], F32, kind="ExternalInput").ap()
    wkT = nc.dram_tensor("wkT", [D, COLS], F32, kind="ExternalInput").ap()
    wvT = nc.dram_tensor("wvT", [D, COLS], F32, kind="ExternalInput").ap()
    woT = nc.dram_tensor("woT", [COLS, D], F32, kind="ExternalInput").ap()
    bq = nc.dram_tensor("bq", [COLS], F32, kind="ExternalInput").ap()
    bk = nc.dram_tensor("bk", [COLS], F32, kind="ExternalInput").ap()
    bv = nc.dram_tensor("bv", [COLS], F32, kind="ExternalInput").ap()
    mb = nc.dram_tensor("mb", [NB, S], F32, kind="ExternalInput").ap()
    onesb = nc.dram_tensor("onesb", [P, P], F32, kind="ExternalInput").ap()

    kT_o = nc.dram_tensor("kT", [COLS, TOK], F32, kind="ExternalOutput").ap()
    v_o = nc.dram_tensor("v", [TOK, COLS], F32, kind="ExternalOutput").ap()
    po_o = nc.dram_tensor("po", [TOK, D], F32, kind="ExternalOutput").ap()

    xT_t = xT.rearrange("(dc p) t -> p dc t", p=P)
    xaT_t = xaT.rearrange("(dc p) t -> p dc t", p=P)
    wqT_t = wqT.rearrange("(dc p) c -> p dc c", p=P)
    wkT_t = wkT.rearrange("(dc p) c -> p dc c", p=P)
    wvT_t = wvT.rearrange("(dc p) c -> p dc c", p=P)
    woT_t = woT.rearrange("(cc p) e -> p cc e", p=P)
    kT_t = kT_o.rearrange("(cc p) t -> p cc t", p=P)
    v_t = v_o.rearrange("(tc p) c -> p tc c", p=P)
    po_t = po_o.rearrange("(tc p) e -> p tc e", p=P)

    with tile.TileContext(nc) as tc:
        for _ in range(reps):
            _emit(tc, nc, dict(
                xT=xT_t, xaT=xaT_t, wqT=wqT_t, wkT=wkT_t, wvT=wvT_t,
                woT=woT_t, bq=bq, bk=bk, bv=bv, mb=mb, onesb=onesb,
                kT=kT_t, v=v_t, po=po_t,
            ))
    nc.compile()
    return nc


def _emit(tc, nc, io):
    from contextlib import ExitStack
    ctx = ExitStack()
    with ctx:
        # ---- persistent SBUF (pools opened lazily to cap concurrent footprint) ----
        pkv = ctx.enter_context(tc.tile_pool(name="pkv", bufs=1))
        kt_sb = pkv.tile([P, CCH, TOK], F32R, tag="kt")         # K^T  4MB
        v_sb = pkv.tile([P, TCH, HEADS, HD + 1], F32R, tag="v")  # V (+ones col)

        const = ctx.enter_context(tc.tile_pool(name="const", bufs=1))
        ones_row = const.tile([1, P], F32R, tag="ones_row")
        bq_sb = const.tile([P, CCH], F32, tag="bq")
        bk_sb = const.tile([P, CCH], F32, tag="bk")
        bv_sb = const.tile([1, COLS], F32R, tag="bv")
        mb_sb = const.tile([P, NB, KVCH], F32, tag="mb")

        nc.sync.dma_start(ones_row[:], io["onesb"].bitcast(F32R)[0:1, :])
        # fused denominator column of V (128*16*8 == 128*128 ones)
        nc.sync.dma_start(v_sb[:, :, :, HD], io["onesb"].bitcast(F32R))
        nc.sync.dma_start(bq_sb[:], io["bq"].rearrange("(cc p) -> p cc", p=P))
        nc.sync.dma_start(bk_sb[:], io["bk"].rearrange("(cc p) -> p cc", p=P))
        nc.sync.dma_start(bv_sb[:], io["bv"].bitcast(F32R)[None, :])
        nc.sync.dma_start(mb_sb[:], io["mb"].rearrange("b (kc p) -> p b kc", p=P))

        # ---- phase 1: K^T and V from xaT ----
        with tc.tile_pool(name="wkv", bufs=1) as wpool, \
             tc.tile_pool(name="xin1", bufs=2) as xpool, \
             tc.tile_pool(name="ps1", bufs=2, space="PSUM") as ps1:
            wk_sb = wpool.tile([P, DCH, COLS], F32R, tag="wk")
            wv_sb = wpool.tile([P, DCH, COLS], F32R, tag="wv")
            nc.sync.dma_start(wk_sb[:], io["wkT"].bitcast(F32R))
            nc.sync.dma_start(wv_sb[:], io["wvT"].bitcast(F32R))
            for tt in range(NTT):
                xa_sb = xpool.tile([P, DCH, TT], F32R, tag="xa")
                nc.sync.dma_start(xa_sb[:], io["xaT"].bitcast(F32R)[:, :, tt * TT:(tt + 1) * TT])
                # K^T tile: [128 cols, 512 tok]
                for cc in range(CCH):
                    ps = ps1.tile([P, TT], F32, tag="pk")
                    for dc in range(DCH):
                        nc.tensor.matmul(
                            ps[:], wk_sb[:, dc, cc * P:(cc + 1) * P],
                            xa_sb[:, dc, :],
                            start=(dc == 0), stop=(dc == DCH - 1))
                    nc.scalar.activation(
                        kt_sb[:, cc, tt * TT:(tt + 1) * TT], ps[:], IDENT,
                        bias=bk_sb[:, cc:cc + 1])
                # V tiles: [128 tok, 512 cols] for 4 tok chunks
                for j in range(TT // P):
                    tcid = tt * (TT // P) + j
                    ps = ps1.tile([P, COLS], F32, tag="pv")
                    for dc in range(DCH):
                        nc.tensor.matmul(
                            ps[:], xa_sb[:, dc, j * P:(j + 1) * P],
                            wv_sb[:, dc, :],
                            start=(dc == 0), stop=False)
                    # + bias via rank-1 matmul: ones[1,128]^T @ bv[1,512]
                    nc.tensor.matmul(ps[:], ones_row[:, :], bv_sb[:, :],
                                     start=False, stop=True)
                    nc.any.tensor_copy(
                        out=v_sb[:, tcid, :, 0:HD],
                        in_=ps[:].rearrange("p (h d) -> p h d", h=HEADS))
                    nc.sync.dma_start(io["v"][:, tcid, :],
                                      v_sb[:, tcid, :, 0:HD].bitcast(F32))
            # K^T cache out
            for cc in range(CCH):
                nc.sync.dma_start(io["kT"][:, cc, :], kt_sb[:, cc, :].bitcast(F32))

        # ---- phase 2: Q^T from xT ----
        pqt = ctx.enter_context(tc.tile_pool(name="pqt", bufs=1))
        qt_sb = pqt.tile([P, CCH, TOK], F32R, tag="qt")         # Q^T  4MB
        with tc.tile_pool(name="wq", bufs=1) as wpool, \
             tc.tile_pool(name="xin2", bufs=2) as xpool, \
             tc.tile_pool(name="ps2", bufs=2, space="PSUM") as ps2:
            wq_sb = wpool.tile([P, DCH, COLS], F32R, tag="wq")
            nc.sync.dma_start(wq_sb[:], io["wqT"].bitcast(F32R))
            for tt in range(NTT):
                x_sb = xpool.tile([P, DCH, TT], F32R, tag="x")
                nc.sync.dma_start(x_sb[:], io["xT"].bitcast(F32R)[:, :, tt * TT:(tt + 1) * TT])
                for cc in range(CCH):
                    ps = ps2.tile([P, TT], F32, tag="pq")
                    for dc in range(DCH):
                        nc.tensor.matmul(
                            ps[:], wq_sb[:, dc, cc * P:(cc + 1) * P],
                            x_sb[:, dc, :],
                            start=(dc == 0), stop=(dc == DCH - 1))
                    nc.scalar.activation(
                        qt_sb[:, cc, tt * TT:(tt + 1) * TT], ps[:], IDENT,
                        bias=bq_sb[:, cc:cc + 1])

        # ---- phase 3: attention per (batch, head, q-tile) ----
        pat = ctx.enter_context(tc.tile_pool(name="pat", bufs=1))
        at_sb = pat.tile([P, CCH, TOK], F32R, tag="at")         # attn^T (normalized)
        with tc.tile_pool(name="probs", bufs=2) as ppool, \
             tc.tile_pool(name="sm", bufs=3) as smpool, \
             tc.tile_pool(name="ps_s", bufs=2, space="PSUM") as ps_s, \
             tc.tile_pool(name="ps_a", bufs=2, space="PSUM") as ps_a:
            for b in range(NB):
                for h in range(HEADS):
                    hp, cc = h % 2, h // 2
                    r0 = hp * HD
                    kt_h = kt_sb[r0:r0 + HD, cc, b * S:(b + 1) * S]
                    qt_h = qt_sb[r0:r0 + HD, cc, b * S:(b + 1) * S]
                    for q in range(NQT):
                        probs = ppool.tile([P, KVCH, QT], F32R, tag="probs")
                        for kc in range(KVCH):
                            sps = ps_s.tile([P, QT], F32, tag="sps")
                            nc.tensor.matmul(
                                sps[:], kt_h[:, kc * P:(kc + 1) * P],
                                qt_h[:, q * QT:(q + 1) * QT],
                                start=True, stop=True)
                            nc.scalar.activation(
                                probs[:, kc, :], sps[:], EXP,
                                bias=mb_sb[:, b, kc:kc + 1], scale=0.125)
                        aps = ps_a.tile([P, QT], F32, tag="aps")
                        for kc in range(KVCH):
                            nc.tensor.matmul(
                                aps[:HD + 1, :],
                                v_sb[:, b * KVCH + kc, h, :],
                                probs[:, kc, :],
                                start=(kc == 0), stop=(kc == KVCH - 1))
                        # normalize: recip of den row, broadcast, multiply
                        rt = smpool.tile([P, QT], F32, tag="rt")
                        nc.vector.reciprocal(rt[HD:HD + 1, :], aps[HD:HD + 1, :])
                        # hop recip row to partition 0 (gpsimd broadcast
                        # sources from partition 0 of its input AP on HW)
                        rt0 = smpool.tile([1, QT], F32, tag="rt0")
                        nc.sync.dma_start(rt0[:], rt[HD:HD + 1, :])
                        rb = smpool.tile([HD, QT], F32, tag="rb")
                        nc.gpsimd.partition_broadcast(rb[:], rt0[:])
                        an = smpool.tile([HD, QT], F32R, tag="an")
                        nc.vector.tensor_tensor(an[:], aps[:HD, :], rb[:], MULT)
                        nc.sync.dma_start(
                            at_sb[r0:r0 + HD, cc,
                                  b * S + q * QT:b * S + (q + 1) * QT],
                            an[:])

        # ---- phase 4: output projection partial ----
        with tc.tile_pool(name="wo", bufs=1) as wpool, \
             tc.tile_pool(name="oout", bufs=3) as opool, \
             tc.tile_pool(name="ps4", bufs=2, space="PSUM") as ps4:
            wo_sb = wpool.tile([P, CCH, D], F32R, tag="wo")
            nc.sync.dma_start(wo_sb[:], io["woT"].bitcast(F32R))
            for tcid in range(TCH):
                for e in range(D // 512):
                    ps = ps4.tile([P, 512], F32, tag="po")
                    for cc in range(CCH):
                        nc.tensor.matmul(
                            ps[:], at_sb[:, cc, tcid * P:(tcid + 1) * P],
                            wo_sb[:, cc, e * 512:(e + 1) * 512],
                            start=(cc == 0), stop=(cc == CCH - 1))
                    ot = opool.tile([P, 512], F32, tag="ot")
                    nc.any.tensor_copy(out=ot[:], in_=ps[:])
                    nc.sync.dma_start(io["po"][:, tcid, e * 512:(e + 1) * 512], ot[:])


def _prep_inputs(inputs):
    x = np.ascontiguousarray(np.asarray(inputs["x"], dtype=np.float32))
    xa = np.ascontiguousarray(np.asarray(inputs["xa"], dtype=np.float32))
    mask = np.asarray(inputs["cross_attn_mask"]).astype(bool).reshape(B_TOT, S)
    wq = np.asarray(inputs["wq"], dtype=np.float32)
    wk = np.asarray(inputs["wk"], dtype=np.float32)
    wv = np.asarray(inputs["wv"], dtype=np.float32)
    wo = np.asarray(inputs["wo"], dtype=np.float32)
    bq = np.ascontiguousarray(np.asarray(inputs["bq"], dtype=np.float32))
    bk = np.ascontiguousarray(np.asarray(inputs["bk"], dtype=np.float32))
    bv = np.ascontiguousarray(np.asarray(inputs["bv"], dtype=np.float32))

    mbias = np.where(mask, np.float32(0.0), np.float32(-1e30)).astype(np.float32)

    in_maps = []
    for c in range(8):
        g, p = c % GROUPS, c // GROUPS
        cs = slice(g * COLS, (g + 1) * COLS)
        bs = slice(p * NB, p * NB + NB)
        in_maps.append({
            "xT": np.ascontiguousarray(x[bs].reshape(TOK, D).T),
            "xaT": np.ascontiguousarray(xa[bs].reshape(TOK, D).T),
            "wqT": np.ascontiguousarray(wq[cs, :].T),
            "wkT": np.ascontiguousarray(wk[cs, :].T),
            "wvT": np.ascontiguousarray(wv[cs, :].T),
            "woT": np.ascontiguousarray(wo[:, cs].T),
            "bq": np.ascontiguousarray(bq[cs]),
            "bk": np.ascontiguousarray(bk[cs]),
            "bv": np.ascontiguousarray(bv[cs]),
            "mb": np.ascontiguousarray(mbias[bs]),
            "onesb": np.ones((P, P), np.float32),
        })
    return in_maps


def _assemble(results, inputs):
    bo = np.asarray(inputs["bo"], dtype=np.float32)
    out = np.zeros((B_TOT, S, D), np.float32)
    kc_out = np.empty((B_TOT, H_TOT, S, HD), np.float32)
    vc_out = np.empty((B_TOT, H_TOT, S, HD), np.float32)
    for c in range(8):
        g, p = c % GROUPS, c // GROUPS
        hs = slice(g * HEADS, (g + 1) * HEADS)
        bs = slice(p * NB, p * NB + NB)
        r = results[c]
        kT = r["kT"]            # [512 c, 2048 tok]
        kc_out[bs, hs] = kT.reshape(HEADS, HD, NB, S).transpose(2, 0, 3, 1)
        v = r["v"]              # [2048 tok, 512 c]
        vc_out[bs, hs] = v.reshape(NB, S, HEADS, HD).transpose(0, 2, 1, 3)
        out[bs] += r["po"].reshape(NB, S, D)
    out += bo
    return out, kc_out, vc_out


def kernel(**inputs):
    global _compiled
    if _compiled is None:
        _compiled = build_nc()
    nc = _compiled
    in_maps = _prep_inputs(inputs)
    res = run_bass_kernel_spmd(nc, in_maps, core_ids=list(range(8)))
    return _assemble(res.results, inputs)
